# revision 10
# baseline (speedup 1.0000x reference)
# GraphSAGE (3-layer, mean aggregation) on 8 Trainium2 NeuronCores.
#
# Sharding: nodes are split into 8 contiguous ranges (6250 per core); edges are
# partitioned by destination node so each core's scatter-adds stay local.  Each
# layer's input features are replicated to every core via AllGather (the x table
# for layer 0 is simply fed to every core), so the per-edge source gathers are
# local HBM reads.
#
# Per-core layer pipeline, for each chunk of 128 destination nodes:
#   - dma_gather the (dst-sorted) edges' source rows from the bf16 feature
#     table in HBM into SBUF  (int16 gather indices => the table is addressed
#     as two halves of 25000 rows)
#   - build a selection matrix S[e, j] = (dst_local[e] == j) * 1/deg per
#     128-edge tile with a single tensor_scalar (is_equal, mult) against an
#     iota row, and accumulate  aggrT += msg_tile^T-contract S  on the PE into
#     PSUM, which yields the transposed mean aggregation [feat, node].
#   - transpose the chunk's own rows with the PE, then
#     h = relu(aggrT^T @ w_l + own^T^T @ w_r (+ b))  with both matmuls
#     accumulating into one PSUM tile; ScalarE applies the relu while
#     draining PSUM.
#   - write h rows to the core's own chunk of the next layer table.
# Between layers: AllGather of the 6250-row chunks -> full 50000-row table.
import os
import sys

import numpy as np

for _p in ("/opt/trn_rl_repo", "/root/.axon_site/_ro/trn_rl_repo"):
    if _p not in sys.path and os.path.isdir(_p):
        sys.path.append(_p)

import ml_dtypes  # noqa: E402

from concourse import bacc, bass, mybir, tile  # noqa: E402
from concourse.bass_utils import axon_active, run_bass_kernel_spmd  # noqa: E402
from concourse.masks import make_identity  # noqa: E402

P = 128
BF16 = mybir.dt.bfloat16
F32 = mybir.dt.float32
I16 = mybir.dt.int16
NP_BF16 = ml_dtypes.bfloat16


class GSCfg:
    """Static problem configuration (shapes shared by all cores)."""

    def __init__(self, n_nodes, n_cores, d_in, d_hid, d_out, half):
        assert n_nodes % n_cores == 0
        self.N = n_nodes
        self.NCORES = n_cores
        self.NPC = n_nodes // n_cores  # nodes per core
        self.D_IN = d_in
        self.D_HID = d_hid
        self.D_OUT = d_out
        self.HALF = half  # gather-table split point (int16 indices)
        assert half < 2**15 and (n_nodes - half) < 2**15
        self.NCH = (self.NPC + P - 1) // P  # dst chunks per core


def preprocess(cfg: GSCfg, src: np.ndarray, dst: np.ndarray):
    """Partition + sort edges by destination, pad to an SPMD-uniform tile
    layout, and build the per-core SBUF-layout index/selection arrays."""
    N, NPC, NCH, HALF = cfg.N, cfg.NPC, cfg.NCH, cfg.HALF
    NC = cfg.NCORES

    deg = np.bincount(dst, minlength=N)
    invdeg_per_node = (1.0 / np.maximum(deg, 1)).astype(np.float32)

    order = np.argsort(dst, kind="stable")
    s_src = src[order]
    s_dst = dst[order]

    # edge index ranges for (core, chunk) and (core, chunk, half)
    # chunk c of core i covers dst in [i*NPC + c*P, i*NPC + min((c+1)*P, NPC))
    chunk_lo = np.empty((NC, NCH), dtype=np.int64)
    chunk_hi = np.empty((NC, NCH), dtype=np.int64)
    for i in range(NC):
        for c in range(NCH):
            d0 = i * NPC + c * P
            d1 = i * NPC + min((c + 1) * P, NPC)
            chunk_lo[i, c] = np.searchsorted(s_dst, d0, side="left")
            chunk_hi[i, c] = np.searchsorted(s_dst, d1, side="left")

    # split each chunk's edges into low-src / high-src groups
    lo_cnt = np.zeros((NC, NCH), dtype=np.int64)
    hi_cnt = np.zeros((NC, NCH), dtype=np.int64)
    for i in range(NC):
        for c in range(NCH):
            e0, e1 = chunk_lo[i, c], chunk_hi[i, c]
            half_sel = (s_src[e0:e1] % NPC) // (NPC // 2)
            lo_cnt[i, c] = int(np.count_nonzero(half_sel == 0))
            hi_cnt[i, c] = (e1 - e0) - lo_cnt[i, c]

    cdiv = lambda a, b: -(-a // b)
    T_low = [int(cdiv(int(lo_cnt[:, c].max()), P)) for c in range(NCH)]
    T_high = [int(cdiv(int(hi_cnt[:, c].max()), P)) for c in range(NCH)]
    TT = sum(T_low) + sum(T_high)
    SLOTS = TT * P

    idx16 = np.zeros((NC, SLOTS), dtype=np.int16)
    dstloc = np.full((NC, SLOTS), -1.0, dtype=np.float32)
    invd = np.zeros((NC, SLOTS), dtype=np.float32)

    # Gather tables are stored in "core-major half" order so the two
    # inter-layer AllGathers (one per half, <1MB per rank => mesh algorithm)
    # write them directly:  node g (core i, local row r) lives at table row
    #   (r // HALFR)*HALF + i*HALFR + (r % HALFR),  HALFR = NPC//2.
    HALFR = NPC // 2
    for i in range(NC):
        pos = 0
        for c in range(NCH):
            e0, e1 = chunk_lo[i, c], chunk_hi[i, c]
            seg_src = s_src[e0:e1]
            seg_dst = s_dst[e0:e1]
            s_i, s_r = seg_src // NPC, seg_src % NPC
            s_half = s_r // HALFR
            s_tidx = s_i * HALFR + (s_r % HALFR)  # row within the half-table
            base = i * NPC + c * P
            for half_sel, t_pad in ((0, T_low[c]), (1, T_high[c])):
                m = s_half == half_sel
                ss = s_tidx[m]
                dd = seg_dst[m]
                n = len(ss)
                assert n <= t_pad * P
                idx16[i, pos : pos + n] = ss.astype(np.int16)
                dstloc[i, pos : pos + n] = (dd - base).astype(np.float32)
                invd[i, pos : pos + n] = invdeg_per_node[dd]
                pos += t_pad * P
        assert pos == SLOTS

    # SBUF layouts:
    #  idx16_sb [128, SLOTS//16]: per gather segment, slot j -> [j%16, j//16],
    #    replicated across the eight 16-partition groups.  Segments are
    #    multiples of 128 slots, so the per-segment wrap equals a global wrap.
    idx_w = idx16.reshape(NC, SLOTS // 16, 16).transpose(0, 2, 1)  # [NC,16,cols]
    idx16_sb = np.ascontiguousarray(np.tile(idx_w, (1, 8, 1)))  # [NC,128,cols]
    #  dstloc/invd [128, TT]: slot j -> [j%128, j//128]
    dstloc_sb = np.ascontiguousarray(dstloc.reshape(NC, TT, P).transpose(0, 2, 1))
    invd_sb = np.ascontiguousarray(invd.reshape(NC, TT, P).transpose(0, 2, 1))

    return T_low, T_high, idx16_sb, dstloc_sb, invd_sb


def table_permute(cfg: GSCfg, x: np.ndarray) -> np.ndarray:
    """Reorder node rows into the core-major-half gather-table layout."""
    g = np.arange(cfg.N)
    i, r = g // cfg.NPC, g % cfg.NPC
    halfr = cfg.NPC // 2
    gp = (r // halfr) * cfg.HALF + i * halfr + (r % halfr)
    out = np.empty_like(x)
    out[gp] = x[g]
    return out


def build_program(cfg: GSCfg, T_low, T_high, has_bias, n_gather_queues=1,
                  skip_collectives=False, skip_gather=False, skip_compute=False,
                  seq_gather=False, maxi=512, repeat=1):
    """Build the SPMD Bass program (identical instruction stream per core)."""
    N, NPC, NCH, HALF = cfg.N, cfg.NPC, cfg.NCH, cfg.HALF
    D_IN, D_HID, D_OUT = cfg.D_IN, cfg.D_HID, cfg.D_OUT
    TT = sum(T_low) + sum(T_high)
    TMAX = max(T_low[c] + T_high[c] for c in range(NCH))

    nc = bacc.Bacc(
        "TRN2",
        target_bir_lowering=False,
        debug=not axon_active(),
        num_devices=cfg.NCORES,
        num_swdge_queues=n_gather_queues,
    )

    xtab = nc.dram_tensor("xtab", [N, D_IN], BF16, kind="ExternalInput")
    xown = nc.dram_tensor("xown", [NPC, D_IN], BF16, kind="ExternalInput")
    idx_d = nc.dram_tensor("idx16", [P, TT * 8], I16, kind="ExternalInput")
    dst_d = nc.dram_tensor("dstloc", [P, TT], F32, kind="ExternalInput")
    inv_d = nc.dram_tensor("invd", [P, TT], F32, kind="ExternalInput")
    w_d = {}
    for li, (din, dout) in enumerate(((D_IN, D_HID), (D_HID, D_HID), (D_HID, D_OUT))):
        w_d[f"wl{li}"] = nc.dram_tensor(f"wl{li}", [din, dout], BF16, kind="ExternalInput")
        w_d[f"wr{li}"] = nc.dram_tensor(f"wr{li}", [din, dout], BF16, kind="ExternalInput")
        if has_bias:
            w_d[f"b{li}"] = nc.dram_tensor(f"b{li}", [P, dout], F32, kind="ExternalInput")
    out_d = nc.dram_tensor("out", [NPC, D_OUT], F32, kind="ExternalOutput")

    from contextlib import ExitStack

    with tile.TileContext(nc) as tc, ExitStack() as stk:
        # ---- constants / static SBUF residents ----
        const = stk.enter_context(tc.tile_pool(name="const", bufs=1))
        iota_i = const.tile([P, P], mybir.dt.int32, name="iota_i")
        nc.gpsimd.iota(iota_i[:], pattern=[[1, P]], base=0, channel_multiplier=0)
        iota_f = const.tile([P, P], F32, name="iota_f")
        nc.vector.tensor_copy(iota_f[:], iota_i[:])
        ident = const.tile([P, P], BF16, name="ident")
        make_identity(nc, ident[:])

        idx_t = const.tile([P, TT * 8], I16, name="idx_t")
        nc.sync.dma_start(idx_t[:], idx_d[:])
        dst_t = const.tile([P, TT], F32, name="dst_t")
        nc.sync.dma_start(dst_t[:], dst_d[:])
        inv_t = const.tile([P, TT], F32, name="inv_t")
        nc.sync.dma_start(inv_t[:], inv_d[:])

        w_t = {}
        for k, d in w_d.items():
            if k == "out":
                continue
            w_t[k] = const.tile(list(d.shape), d.dtype, name=f"{k}_t")
            nc.sync.dma_start(w_t[k][:], d[:])

        # ---- inter-layer DRAM tables ----
        dram = stk.enter_context(tc.tile_pool(name="dram", bufs=1, space="DRAM"))

        # ---- working pools ----
        msgp = stk.enter_context(tc.tile_pool(name="msg", bufs=3))
        sp = stk.enter_context(tc.tile_pool(name="sel", bufs=4))
        wk = stk.enter_context(tc.tile_pool(name="wk", bufs=3))
        ps_ag = stk.enter_context(tc.tile_pool(name="ps_ag", bufs=2, space="PSUM"))
        ps_tr = stk.enter_context(tc.tile_pool(name="ps_tr", bufs=2, space="PSUM"))
        ps_h = stk.enter_context(tc.tile_pool(name="ps_h", bufs=2, space="PSUM"))

        gq = [0]
        # The SWDGE descriptor ring holds dynamic_dma_scratch_size/16 = 1024
        # descriptors per queue; one instruction must fit entirely, so cap the
        # per-instruction index count well below that.
        MAXI = maxi

        def gather(out_ap, tab_ap, col0, n_idx):
            if skip_gather:
                return
            if seq_gather:
                # bandwidth-floor variant: same volume, contiguous rows
                for t in range(n_idx // P):
                    nc.sync.dma_start(out_ap[:, t, :], tab_ap[t * P : (t + 1) * P, :])
                return
            for off in range(0, n_idx, MAXI):
                n = min(MAXI, n_idx - off)
                t0, t1 = off // P, (off + n) // P
                nc.gpsimd.dma_gather(
                    out_ap[:, t0:t1, :],
                    tab_ap,
                    idx_t[:, col0 + off // 16 : col0 + (off + n) // 16],
                    num_idxs=n,
                    num_idxs_reg=n,
                    elem_size=D_HID,
                    queue_num=gq[0] % n_gather_queues,
                )
                gq[0] += 1

        for rep in range(repeat):
         h_own = [
             dram.tile([NPC, D_HID], BF16, name=f"h_own{li}_r{rep}")
             for li in range(2)
         ]
         h_full = [
             tuple(
                 dram.tile([HALF, D_HID], BF16, name=f"h_full{li}_{hh}_r{rep}",
                           addr_space="Shared")
                 for hh in range(2)
             )
             for li in range(2)
         ]
         out_t = out_d if rep == repeat - 1 else dram.tile(
             [NPC, D_OUT], F32, name=f"oscr_r{rep}"
         )
         for layer in range(3):
            din = D_IN if layer == 0 else D_HID
            dout = D_HID if layer < 2 else D_OUT
            if layer == 0:
                tab_lo, tab_hi = xtab[0:HALF, :], xtab[HALF:N, :]
                own = xown[:]
            else:
                tab_lo, tab_hi = h_full[layer - 1][0][:], h_full[layer - 1][1][:]
                own = h_own[layer - 1][:]
            wl_t = w_t[f"wl{layer}"]
            wr_t = w_t[f"wr{layer}"]
            col = 0
            til = 0
            for c in range(NCH):
                Tl, Th = T_low[c], T_high[c]
                T = Tl + Th
                nrows = min(P, NPC - c * P)

                msg_t = msgp.tile([P, TMAX, din], BF16, tag="msg")
                if Tl:
                    gather(msg_t[:, :Tl, :], tab_lo, col, Tl * P)
                if Th:
                    gather(msg_t[:, Tl:T, :], tab_hi, col + Tl * 8, Th * P)
                col += T * 8

                if skip_compute:
                    til += T
                    if layer < 2:
                        h_sb = wk.tile([P, dout], BF16, tag="h_sb")
                        nc.vector.memset(h_sb[:], 0)
                        nc.sync.dma_start(
                            h_own[layer][c * P : c * P + nrows, :], h_sb[:nrows]
                        )
                    else:
                        o_sb = wk.tile([P, dout], F32, tag="o_sb")
                        nc.vector.memset(o_sb[:], 0)
                        nc.sync.dma_start(out_t[c * P : c * P + nrows, :], o_sb[:nrows])
                    continue

                # transposed mean aggregation accumulated on the PE
                agg_ps = ps_ag.tile([P, P], F32, tag="agg")
                for t in range(T):
                    s_t = sp.tile([P, P], BF16, tag="S")
                    nc.vector.tensor_scalar(
                        s_t[:],
                        iota_f[:],
                        dst_t[:, til + t : til + t + 1],
                        inv_t[:, til + t : til + t + 1],
                        mybir.AluOpType.is_equal,
                        mybir.AluOpType.mult,
                    )
                    nc.tensor.matmul(
                        agg_ps[:],
                        lhsT=msg_t[:, t, :],
                        rhs=s_t[:],
                        start=(t == 0),
                        stop=(t == T - 1),
                    )
                til += T
                aggT = wk.tile([P, P], BF16, tag="aggT")
                nc.vector.tensor_copy(aggT[:], agg_ps[:])

                # own-rows transpose (for the root-weight matmul)
                own_sb = wk.tile([P, din], BF16, tag="own")
                if nrows < P:
                    nc.vector.memset(own_sb[:], 0)
                nc.sync.dma_start(own_sb[:nrows], own[c * P : c * P + nrows, :])
                xT_ps = ps_tr.tile([P, P], BF16, tag="xT")
                nc.tensor.transpose(xT_ps[:], own_sb[:], ident[:])
                xT = wk.tile([P, P], BF16, tag="xT_sb")
                nc.vector.tensor_copy(xT[:], xT_ps[:])

                # h = relu(aggr @ wl + own @ wr (+ b))
                h_ps = ps_h.tile([P, dout], F32, tag="h")
                nc.tensor.matmul(h_ps[:], lhsT=aggT[:], rhs=wl_t[:], start=True, stop=False)
                nc.tensor.matmul(h_ps[:], lhsT=xT[:], rhs=wr_t[:], start=False, stop=True)

                if layer < 2:
                    h_sb = wk.tile([P, dout], BF16, tag="h_sb")
                    if has_bias:
                        nc.vector.tensor_tensor(
                            h_sb[:], h_ps[:], w_t[f"b{layer}"][:], mybir.AluOpType.add
                        )
                        nc.scalar.activation(
                            h_sb[:], h_sb[:], mybir.ActivationFunctionType.Relu
                        )
                    else:
                        nc.scalar.activation(
                            h_sb[:], h_ps[:], mybir.ActivationFunctionType.Relu
                        )
                    nc.sync.dma_start(
                        h_own[layer][c * P : c * P + nrows, :], h_sb[:nrows]
                    )
                else:
                    o_sb = wk.tile([P, dout], F32, tag="o_sb")
                    if has_bias:
                        nc.vector.tensor_tensor(
                            o_sb[:], h_ps[:], w_t[f"b{layer}"][:], mybir.AluOpType.add
                        )
                    else:
                        nc.vector.tensor_copy(o_sb[:], h_ps[:])
                    nc.sync.dma_start(out_t[c * P : c * P + nrows, :], o_sb[:nrows])

            if layer < 2 and not skip_collectives:
                # two sub-1MB AllGathers (mesh algorithm), one per node-half;
                # output order = core-major within each half = table layout
                HALFR = NPC // 2
                for hh in range(2):
                    nc.gpsimd.collective_compute(
                        "AllGather",
                        mybir.AluOpType.bypass,
                        replica_groups=[list(range(cfg.NCORES))],
                        ins=[h_own[layer][hh * HALFR : (hh + 1) * HALFR, :]],
                        outs=[h_full[layer][hh].opt()],
                    )

    nc.compile()
    return nc


def run(cfg: GSCfg, inputs: dict, trace: bool = False, tmpdir: str | None = None):
    """Preprocess, build, and run on the 8 cores; returns (out, results)."""
    x = np.asarray(inputs["x"], dtype=np.float32)
    ei = np.asarray(inputs["edge_index"])
    src = ei[0].astype(np.int64)
    dst = ei[1].astype(np.int64)

    T_low, T_high, idx16_sb, dstloc_sb, invd_sb = preprocess(cfg, src, dst)

    biases = [np.asarray(inputs[f"b{i}"], dtype=np.float32) for i in range(3)]
    has_bias = any(np.any(b != 0) for b in biases)

    nc = build_program(cfg, T_low, T_high, has_bias)

    x_bf = x.astype(NP_BF16)
    xtab = table_permute(cfg, x_bf)
    in_maps = []
    for i in range(cfg.NCORES):
        m = {
            "xtab": xtab,
            "xown": np.ascontiguousarray(x_bf[i * cfg.NPC : (i + 1) * cfg.NPC]),
            "idx16": idx16_sb[i],
            "dstloc": dstloc_sb[i],
            "invd": invd_sb[i],
        }
        for li in range(3):
            m[f"wl{li}"] = np.asarray(inputs[f"w_l{li}"], dtype=np.float32).astype(NP_BF16)
            m[f"wr{li}"] = np.asarray(inputs[f"w_r{li}"], dtype=np.float32).astype(NP_BF16)
            if has_bias:
                m[f"b{li}"] = np.tile(biases[li][None, :], (P, 1))
        in_maps.append(m)

    results = run_bass_kernel_spmd(
        nc,
        in_maps,
        core_ids=list(range(cfg.NCORES)),
        trace=trace,
        tmpdir=tmpdir,
    )
    outs = [np.asarray(r["out"], dtype=np.float32) for r in results.results]
    return np.concatenate(outs, axis=0), results


def kernel(**inputs) -> np.ndarray:
    cfg = GSCfg(n_nodes=50000, n_cores=8, d_in=128, d_hid=128, d_out=64, half=25000)
    out, _ = run(cfg, inputs, trace=False)
    return out


if __name__ == "__main__":
    import jax

    sys.path.insert(0, os.path.dirname(os.path.abspath(__file__)))
    import reference

    inputs = {k: np.asarray(v) for k, v in reference.setup_inputs().items()}
    expected = np.asarray(reference.reference(**inputs))
    actual = kernel(**inputs)
    err = np.abs(actual - expected)
    rel = np.linalg.norm(actual - expected) / np.linalg.norm(expected)
    print("max abs err", err.max(), "rel", rel)

